# revision 1
# baseline (speedup 1.0000x reference)
"""Trainium2 Bass kernel for nn_CustomLoss (CrossEntropy + binary-remap BCE).

loss = mean_i[ logsumexp(pred_i) - pred_i[t_i] ]
     + 100 * mean_i[ 1{ LUT[argmax(pred_i)] != LUT[t_i] } ]

with LUT = [0,0,1,1,1,1,1,1,0,0]  (LUT[j] = 1 iff 2 <= j <= 7).

Sharding: data-parallel over the batch axis across 8 NeuronCores; each core
returns 3 per-partition partial sums which the host folds into the scalar.

The DVE's second SBUF read port is shared with GPSIMD, so 2-port DVE
instructions and GPSIMD instructions destroy each other's throughput.  This
version keeps every DVE instruction single-ported:

  * The host packs each row as 11 floats: [h, pred0+16 .. pred9+16] where
    h = 1000 + 11*w + 1 + t + 0.25*bt  (t = target class, bt = binary
    target).  h dominates every shifted logit and increases with w, so a
    single running-max scan inside the fused gather op recovers the
    current row's target position with no second tensor stream.
  * GATHER_SCAN_ANT (DVE, 1 port): hold = scanmax(x); select x where
    hold - (1000 + k) in [0, 0.5); accumulate  ->  sum of pred'[t].
  * SB_EXTRACT_ANT (DVE, 1 port): sb = 1 - 2*(frac(h) > 0) in {+1,-1}
    from the 0.25*bt fraction, via the +2^23 round trick.
  * mid-6/outer-4 maxes: strided tensor_reduce (1 port), the outer-4 with
    negate=True so GPSIMD (add/mult only) forms d6 = m6 - m4 and
    q = d6 * sb.
  * SUM_GT_ANT (DVE, 1 port): accumulate (q > 0) = binary mismatches.

  DMA   : packed tiles [128, W*11] f32 (contiguous rows)
  ACT   : E = exp(pred' - 16) -> bf16 on the strided logit columns (combined
          Exp/Ln table), Ln of row-sums with per-partition accumulate
  GPSIMD: per-row sum of E via bf16 add tree (10 -> 5 -> 2+1 -> 1), d6, q
  DVE   : the reduces + three single-ported fused ops above
"""

import numpy as np

# ---------------------------------------------------------------- constants
N = 2_000_000
C = 10
CW = 11                       # row width incl. the packed sentinel column
N_CORES = 8
P = 128
# variable tile widths: small first tile to start compute early, small last
# tile to shorten the drain; sum * P = padded rows per core
TILE_WS = [123, 489, 489, 489, 366]
W_SUM = sum(TILE_WS)          # 1,956
ROWS_CORE_PAD = P * W_SUM     # 250,368
ROWS_CORE = N // N_CORES      # 250,000
PAD_PER_CORE = ROWS_CORE_PAD - ROWS_CORE  # 368
SHIFT = 16.0
HBASE = 1000.0

_CACHE = {}


# ------------------------------------------------------- custom DVE ops
def _register_custom_ops():
    """Register the three fused single-port DVE ops (idempotent)."""
    import concourse.dve_ops as dve_ops
    from concourse.dve_spec import (
        Spec, Src0, Zero, One, select, lower, AluOp, Bin, scan, C0, C2,
    )
    from concourse.dve_uop import DveOpSpec

    def _get(name):
        for op in dve_ops.OPS:
            if op.name == name:
                return op
        return None

    def _register(name, spec):
        existing = _get(name)
        if existing is not None:
            return existing
        opcode = dve_ops._CUSTOM_DVE_ROW_BASE + len(dve_ops.OPS)
        assert opcode < 0x20, "custom DVE opcode rows exhausted"
        from concourse.dve_ops import has_src1
        shas = {}
        for ver in ("v3", "v4"):
            uops = lower(spec, ver=ver)
            tmp = DveOpSpec(name=name, opcode=opcode, uops=uops,
                            rd1_en=has_src1(spec))
            shas[ver] = tmp.sha(ver)
        op = dve_ops.DveOp(name, spec, subdim=False, uops_sha=shas)
        dve_ops.OPS.append(op)
        dve_ops._SUB_OPCODE_FOR_NAME[name] = opcode
        dve_ops.CUSTOM_DVE_SPECS[name] = spec
        return op

    # GATHER_SCAN: hold = running max; keep x where hold-(imm2+k) in [0,s0)
    def _gather_ref(in0, in1, s0, s1, imm2):
        p = in0.shape[0]
        x = np.asarray(in0, np.float32).reshape(p, -1)
        hold = np.maximum.accumulate(x, axis=1)
        idxk = np.float32(imm2) + np.arange(x.shape[1],
                                            dtype=np.float32)[None, :]
        diff = hold - idxk
        s0v = np.asarray(s0, np.float32).reshape(p, 1) \
            if isinstance(s0, np.ndarray) else np.float32(s0)
        keep = (diff >= 0) & (diff < s0v)
        out = np.where(keep, x, np.float32(0.0))
        acc = out.sum(axis=1, dtype=np.float64).astype(np.float32)[:, None]
        return out.reshape(in0.shape), acc

    idxk = scan(AluOp.ADD, One, init=Bin(AluOp.SUBTRACT, C2, One))
    hold = scan(AluOp.MAX, Src0)
    diff = hold - idxk
    gather_spec = Spec(
        body=select((diff >= Zero) & (diff < C0), Src0, Zero),
        accum=AluOp.ADD,
        accum_init=Zero,
        reference=_gather_ref,
    )
    gop = _register("GATHER_SCAN_ANT", gather_spec)

    # SB_EXTRACT: sb = 1 - 2*(frac(x) > 0), frac via the +2^23 round trick
    def _sb_ref(in0, in1, s0, s1, imm2):
        p = in0.shape[0]
        x = np.asarray(in0, np.float32).reshape(p, -1)
        r = (x + np.float32(imm2)).astype(np.float32) - np.float32(imm2)
        fr = x - r.astype(np.float32)
        sb = np.where(fr > 0, np.float32(-1.0), np.float32(1.0))
        return sb.reshape(in0.shape)

    r = (Src0 + C2) - C2
    g = (Src0 - r) > Zero
    sb_spec = Spec(
        body=(One - g) - g,
        reference=_sb_ref,
    )
    sop = _register("SB_EXTRACT_ANT", sb_spec)

    # SUM_GT: accum += (x > 0)
    def _gt_ref(in0, in1, s0, s1, imm2):
        p = in0.shape[0]
        x = np.asarray(in0, np.float32).reshape(p, -1)
        out = (x > 0).astype(np.float32)
        acc = out.sum(axis=1, dtype=np.float64).astype(np.float32)[:, None]
        return out.reshape(in0.shape), acc

    gt_spec = Spec(
        body=Src0 > Zero,
        accum=AluOp.ADD,
        accum_init=Zero,
        reference=_gt_ref,
    )
    qop = _register("SUM_GT_ANT", gt_spec)
    return gop, sop, qop


# ------------------------------------------------------------- device build
def _build_nc(tile_ws=None):
    import concourse.bass as bass
    import concourse.tile as tile
    from concourse import bacc, mybir

    gop, sop, qop = _register_custom_ops()
    f32 = mybir.dt.float32
    i32 = mybir.dt.int32
    bf16 = mybir.dt.bfloat16
    A = mybir.ActivationFunctionType
    X = mybir.AxisListType.X
    XY = mybir.AxisListType.XY
    alu = mybir.AluOpType

    if tile_ws is None:
        tile_ws = TILE_WS
    tiles = len(tile_ws)
    nc = bacc.Bacc("TRN2", target_bir_lowering=False, debug=False,
                   num_devices=N_CORES)
    comb_ds = [
        nc.dram_tensor(f"comb{i}", [P, wi * CW], f32,
                       kind="ExternalInput").ap()
        for i, wi in enumerate(tile_ws)
    ]
    out_d = nc.dram_tensor("out", [P, 3], f32, kind="ExternalOutput").ap()

    with tile.TileContext(nc) as tc:
        with (
            tc.tile_pool(name="io", bufs=3) as io,
            tc.tile_pool(name="ep", bufs=3) as ep,
            tc.tile_pool(name="zp", bufs=2) as zp,
            tc.tile_pool(name="mp", bufs=2) as mp,
            tc.tile_pool(name="cp", bufs=1) as cp,
        ):
            bias16 = cp.tile([P, 1], f32)
            nc.gpsimd.memset(bias16[:], -SHIFT)
            acc_all = cp.tile([P, 3, tiles], f32)
            acc_lg = acc_all[:, 0, :]
            acc_g = acc_all[:, 1, :]
            acc_mm = acc_all[:, 2, :]

            for i in range(tiles):
                w = tile_ws[i]
                ct = io.tile([P, w * CW], f32, tag="comb")
                nc.sync.dma_start(ct[:], comb_ds[i])
                cv = ct[:].rearrange("p (w x) -> p w x", x=CW)
                v10 = cv[:, :, 1:CW]

                # ---- CE path: exp on ACT, row-sum tree on GPSIMD, ln on ACT
                et = ep.tile([P, w * C], bf16, tag="E")
                nc.scalar.activation(et[:], v10, A.Exp, bias=bias16[:])

                e3 = et[:].rearrange("p (w c) -> p w c", c=C)
                z1 = zp.tile([P, w, 5], bf16, tag="z1")
                nc.gpsimd.tensor_tensor(z1[:], e3[:, :, 0:5], e3[:, :, 5:10],
                                        op=alu.add)
                z2 = zp.tile([P, w, 2], bf16, tag="z2")
                nc.gpsimd.tensor_tensor(z2[:], z1[:, :, 0:2], z1[:, :, 2:4],
                                        op=alu.add)
                z3 = zp.tile([P, w], bf16, tag="z3")
                nc.gpsimd.tensor_tensor(z3[:], z2[:, :, 0], z2[:, :, 1],
                                        op=alu.add)
                s = zp.tile([P, w], bf16, tag="s")
                nc.gpsimd.tensor_tensor(s[:], z3[:], z1[:, :, 4], op=alu.add)

                lg = zp.tile([P, w], f32, tag="lg")
                nc.scalar.activation(lg[:], s[:], A.Ln,
                                     accum_out=acc_lg[:, i:i + 1])

                # ---- BCE path: group max reduces on DVE (1 port each)
                m6 = mp.tile([P, w], f32, tag="m6")
                nc.vector.reduce_max(m6[:], cv[:, :, 3:9], axis=X)
                v4 = v10.rearrange("p w (g e) -> p w g e", g=5, e=2)
                m4n = mp.tile([P, w], f32, tag="m4n")
                nc.vector.reduce_max(m4n[:], v4[:, :, 0:5:4, :], axis=XY,
                                     negate=True)
                sb = mp.tile([P, w], f32, tag="sb")
                nc.vector._custom_dve(
                    sop, out=sb[:], in0=cv[:, :, 0], imm2=8388608.0)

                # d6 = m6 - m4, q = d6 * sb on GPSIMD (add/mult only)
                d6 = mp.tile([P, w], f32, tag="d6")
                nc.gpsimd.tensor_tensor(d6[:], m6[:], m4n[:], op=alu.add)
                q = mp.tile([P, w], f32, tag="q")
                nc.gpsimd.tensor_tensor(q[:], d6[:], sb[:], op=alu.mult)

                # ---- fused single-port gather + mismatch count (DVE)
                nc.vector._custom_dve(
                    gop, out=ct[:], in0=ct[:], s0=0.5, imm2=HBASE,
                    accum_out=acc_g[:, i:i + 1])
                nc.vector._custom_dve(
                    qop, out=q[:], in0=q[:],
                    accum_out=acc_mm[:, i:i + 1])

            # ---- final per-partition reduction + store (one fused reduce)
            out_t = cp.tile([P, 3], f32)
            nc.vector.reduce_sum(out_t[:], acc_all[:], axis=X)
            nc.sync.dma_start(out_d[:], out_t[:])

    # Force a single activation table containing both Exp and Ln so the
    # compiler does not ping-pong ACT_TABLE_LOADs.  Table ids are positional,
    # so keep the dict shape and empty the other sets.
    import concourse.bacc as bacc_mod
    from concourse.hw_specs import get_activation_tables
    orig = get_activation_tables(nc.m.arch)
    combined = None
    for k, v in orig.items():
        if (mybir.ActivationFunctionType.Exp in v
                and mybir.ActivationFunctionType.Ln in v):
            combined = k
            break
    if combined is not None:
        patched = {k: (v if k == combined else set()) for k, v in orig.items()}
        saved = bacc_mod.get_activation_tables
        bacc_mod.get_activation_tables = lambda arch: patched
        try:
            nc.compile()
        finally:
            bacc_mod.get_activation_tables = saved
    else:
        nc.compile()
    return nc


def _get_nc():
    if "nc" not in _CACHE:
        _CACHE["nc"] = _build_nc()
    return _CACHE["nc"]


# ------------------------------------------------------------------- host
def _host_prep(pred, target):
    """Shard + pad inputs, pack [h, pred+16] rows per core."""
    pred = np.asarray(pred, dtype=np.float32)
    target = np.asarray(target).astype(np.int32)

    in_maps = []
    rows = ROWS_CORE
    for c in range(N_CORES):
        pc = pred[c * rows:(c + 1) * rows] + np.float32(SHIFT)
        tc_ = target[c * rows:(c + 1) * rows]
        if PAD_PER_CORE:
            pc = np.concatenate(
                [pc, np.full((PAD_PER_CORE, C), SHIFT, np.float32)], axis=0)
            tc_ = np.concatenate(
                [tc_, np.zeros(PAD_PER_CORE, np.int32)], axis=0)
        m = {}
        off = 0
        for i, wi in enumerate(TILE_WS):
            n_i = P * wi
            pi = pc[off:off + n_i].reshape(P, wi, C)
            ti = tc_[off:off + n_i].reshape(P, wi)
            off += n_i
            bt = (ti >= 2) & (ti <= 7)
            w_idx = np.arange(wi, dtype=np.float64)[None, :] * CW
            h = (HBASE + w_idx + 1.0 + ti + 0.25 * bt).astype(np.float32)
            comb = np.empty((P, wi, CW), np.float32)
            comb[..., 0] = h
            comb[..., 1:] = pi
            m[f"comb{i}"] = np.ascontiguousarray(comb.reshape(P, wi * CW))
        in_maps.append(m)
    return in_maps


def kernel(pred, target):
    from concourse.bass_utils import run_bass_kernel_spmd

    nc = _get_nc()
    in_maps = _host_prep(pred, target)
    res = run_bass_kernel_spmd(nc, in_maps, core_ids=list(range(N_CORES)))

    sum_lg = 0.0
    sum_g = 0.0
    sum_mm = 0.0
    for c in range(N_CORES):
        o = res.results[c]["out"].astype(np.float64)
        sum_lg += o[:, 0].sum()
        sum_g += o[:, 1].sum()
        sum_mm += o[:, 2].sum()

    # padded rows: pred' = 16 -> logsumexp = ln(10), gather = 16, mismatch 0.
    # every (real and padded) row's gather picks pred + 16.
    sum_lg -= N_CORES * PAD_PER_CORE * np.log(10.0)
    sum_g -= SHIFT * N_CORES * ROWS_CORE_PAD

    ce = (sum_lg - sum_g) / N
    bce = 100.0 * sum_mm / N
    return np.float32(ce + bce)



# revision 3
# speedup vs baseline: 1.2719x; 1.2719x over previous
"""Trainium2 Bass kernel for nn_CustomLoss (CrossEntropy + binary-remap BCE).

loss = mean_i[ logsumexp(pred_i) - pred_i[t_i] ]
     + 100 * mean_i[ 1{ LUT[argmax(pred_i)] != LUT[t_i] } ]

with LUT = [0,0,1,1,1,1,1,1,0,0]  (LUT[j] = 1 iff 2 <= j <= 7).

Sharding: data-parallel over the batch axis across 8 NeuronCores; each core
returns 3 per-partition partial sums which the host folds into the scalar.

Engine plan (per core, ~250K rows):
  DMA   : logits bf16 [P, w*10] in group order [g6 | g4] (20 B/row) and a
          dup stream f32 [P, w] = pred[t] + 64*bt (4 B/row).  24 B/row total.
  ACT   : E = exp(logits) bf16 (contiguous, 1 elem/cycle/lane), Ln of the
          row-sums with per-partition accumulate -> sum of logsumexp.
  DVE   : row-sum reduce [P, w, 10] bf16 -> f32; two group-max reduces
          (m6 = max E[g6], m4n = -max E[g4], bf16 2x packing); dup-stream
          accumulate (tensor_scalar) -> sum of pred[t]; COUNT_MM_ANT custom
          2-port op: mismatch = bt ? (d6 < 0) : (d6 >= 0), accum -> count.
          argmax in exp space is exact (exp monotone); bf16 rounding ties
          resolve like argmax-first-index because g6 is packed first.
  GPSIMD: d6 = m6 + m4n.

The bt flag rides in the dup stream (+64 when LUT[t] = 1, threshold 32);
the +64*sum(bt) is target-derived and subtracted on the host.  Padded rows
have all-zero logits (lse = ln 10, d6 = 0 -> counts once via the bt=0
branch) and dup = 0; both pad contributions are exact host-side constants.
"""

import numpy as np

# ---------------------------------------------------------------- constants
N = 2_000_000
C = 10
N_CORES = 8
P = 128
# variable tile widths: small first tile to start compute early, small last
# tile to shorten the drain; sum * P = padded rows per core
TILE_WS = [123, 489, 489, 489, 366]
W_SUM = sum(TILE_WS)          # 1,956
ROWS_CORE_PAD = P * W_SUM     # 250,368
ROWS_CORE = N // N_CORES      # 250,000
PAD_PER_CORE = ROWS_CORE_PAD - ROWS_CORE  # 368
# class order: LUT=1 group (2..7) first, LUT=0 group (0,1,8,9) second
PERM = [2, 3, 4, 5, 6, 7, 0, 1, 8, 9]
BT_SHIFT = 64.0
BT_THRESH = 32.0

_CACHE = {}


# ------------------------------------------------------- custom DVE op
def _register_custom_ops():
    """Register the 2-port mismatch-count op (idempotent)."""
    import concourse.dve_ops as dve_ops
    from concourse.dve_spec import (
        Spec, Src0, Src1, Zero, select, lower, AluOp, Bin, C0,
    )
    from concourse.dve_uop import DveOpSpec

    def _get(name):
        for op in dve_ops.OPS:
            if op.name == name:
                return op
        return None

    def _register(name, spec):
        existing = _get(name)
        if existing is not None:
            return existing
        opcode = dve_ops._CUSTOM_DVE_ROW_BASE + len(dve_ops.OPS)
        assert opcode < 0x20, "custom DVE opcode rows exhausted"
        from concourse.dve_ops import has_src1
        shas = {}
        for ver in ("v3", "v4"):
            uops = lower(spec, ver=ver)
            tmp = DveOpSpec(name=name, opcode=opcode, uops=uops,
                            rd1_en=has_src1(spec))
            shas[ver] = tmp.sha(ver)
        op = dve_ops.DveOp(name, spec, subdim=False, uops_sha=shas)
        dve_ops.OPS.append(op)
        dve_ops._SUB_OPCODE_FOR_NAME[name] = opcode
        dve_ops.CUSTOM_DVE_SPECS[name] = spec
        return op

    # COUNT_MM: in0 = d6 = m6 - m4, in1 = dup (>= s0 encodes bt = 1).
    # mismatch = bt ? (d6 < 0) : (d6 >= 0); accum counts mismatches.
    def _count_ref(in0, in1, s0, s1, imm2):
        p = in0.shape[0]
        d6 = np.asarray(in0, np.float32).reshape(p, -1)
        dup = np.asarray(in1, np.float32).reshape(p, -1)
        s0v = np.asarray(s0, np.float32).reshape(p, 1) \
            if isinstance(s0, np.ndarray) else np.float32(s0)
        bt = dup >= s0v
        mm = np.where(bt, d6 < 0, d6 >= 0).astype(np.float32)
        acc = mm.sum(axis=1, dtype=np.float64).astype(np.float32)[:, None]
        return mm.reshape(in0.shape), acc

    body = select(Src1 >= C0,
                  Bin(AluOp.IS_LT, Src0, Zero),
                  Bin(AluOp.IS_GE, Src0, Zero))
    count_spec = Spec(
        body=body,
        accum=AluOp.ADD,
        accum_init=Zero,
        reference=_count_ref,
    )
    cop = _register("COUNT_MM_ANT", count_spec)
    return cop


# ------------------------------------------------------------- device build
def _build_nc(tile_ws=None):
    import concourse.bass as bass
    import concourse.tile as tile
    from concourse import bacc, mybir

    cop = _register_custom_ops()
    f32 = mybir.dt.float32
    bf16 = mybir.dt.bfloat16
    A = mybir.ActivationFunctionType
    X = mybir.AxisListType.X
    alu = mybir.AluOpType

    if tile_ws is None:
        tile_ws = TILE_WS
    tiles = len(tile_ws)
    nc = bacc.Bacc("TRN2", target_bir_lowering=False, debug=False,
                   num_devices=N_CORES)
    log_ds = [
        nc.dram_tensor(f"log{i}", [P, wi * C], bf16,
                       kind="ExternalInput").ap()
        for i, wi in enumerate(tile_ws)
    ]
    dup_ds = [
        nc.dram_tensor(f"dup{i}", [P, wi], f32,
                       kind="ExternalInput").ap()
        for i, wi in enumerate(tile_ws)
    ]
    out_d = nc.dram_tensor("out", [P, 3], f32, kind="ExternalOutput").ap()

    with tile.TileContext(nc) as tc:
        with (
            tc.tile_pool(name="io", bufs=3) as io,
            tc.tile_pool(name="ep", bufs=3) as ep,
            tc.tile_pool(name="mp", bufs=2) as mp,
            tc.tile_pool(name="cp", bufs=1) as cp,
        ):
            acc_all = cp.tile([P, 3, tiles], f32)
            acc_lg = acc_all[:, 0, :]
            acc_g = acc_all[:, 1, :]
            acc_mm = acc_all[:, 2, :]

            for i in range(tiles):
                w = tile_ws[i]
                lt = io.tile([P, w * C], bf16, tag="log")
                nc.sync.dma_start(lt[:], log_ds[i])
                dt = io.tile([P, w], f32, tag="dup")
                nc.sync.dma_start(dt[:], dup_ds[i])

                # ---- CE path: exp (ACT), row-sum (DVE), ln+accum (ACT)
                et = ep.tile([P, w * C], bf16, tag="E")
                nc.scalar.activation(et[:], lt[:], A.Exp)
                e3 = et[:].rearrange("p (w c) -> p w c", c=C)

                s = mp.tile([P, w], f32, tag="s")
                nc.vector.reduce_sum(s[:], e3, axis=X)
                lnj = mp.tile([P, w], f32, tag="lnj")
                nc.scalar.activation(lnj[:], s[:], A.Ln,
                                     accum_out=acc_lg[:, i:i + 1])

                # ---- gathered-logit accumulate (DVE tensor_scalar)
                gj = mp.tile([P, w], f32, tag="gj")
                nc.vector.tensor_scalar(gj[:], dt[:], 1.0, 0.0,
                                        op0=alu.mult, op1=alu.add,
                                        accum_out=acc_g[:, i:i + 1])

                # ---- BCE path: group maxes in exp space (DVE, bf16 2x)
                m6 = mp.tile([P, w], f32, tag="m6")
                nc.vector.reduce_max(m6[:], e3[:, :, 0:6], axis=X)
                m4n = mp.tile([P, w], f32, tag="m4n")
                nc.vector.reduce_max(m4n[:], e3[:, :, 6:10], axis=X,
                                     negate=True)
                d6 = mp.tile([P, w], f32, tag="d6")
                nc.gpsimd.tensor_tensor(d6[:], m6[:], m4n[:], op=alu.add)

                cj = mp.tile([P, w], f32, tag="cj")
                nc.vector._custom_dve(
                    cop, out=cj[:], in0=d6[:], in1=dt[:], s0=BT_THRESH,
                    accum_out=acc_mm[:, i:i + 1])

            # ---- final per-partition reduction + store (one fused reduce)
            out_t = cp.tile([P, 3], f32)
            nc.vector.reduce_sum(out_t[:], acc_all[:], axis=X)
            nc.sync.dma_start(out_d[:], out_t[:])

    # Force a single activation table containing both Exp and Ln so the
    # compiler does not ping-pong ACT_TABLE_LOADs.  Table ids are positional,
    # so keep the dict shape and empty the other sets.
    import concourse.bacc as bacc_mod
    from concourse.hw_specs import get_activation_tables
    orig = get_activation_tables(nc.m.arch)
    combined = None
    for k, v in orig.items():
        if (mybir.ActivationFunctionType.Exp in v
                and mybir.ActivationFunctionType.Ln in v):
            combined = k
            break
    if combined is not None:
        patched = {k: (v if k == combined else set()) for k, v in orig.items()}
        saved = bacc_mod.get_activation_tables
        bacc_mod.get_activation_tables = lambda arch: patched
        try:
            nc.compile()
        finally:
            bacc_mod.get_activation_tables = saved
    else:
        nc.compile()
    return nc


def _get_nc():
    if "nc" not in _CACHE:
        _CACHE["nc"] = _build_nc()
    return _CACHE["nc"]


# ------------------------------------------------------------------- host
def _host_prep(pred, target):
    """Shard + pad inputs, pack bf16 logits (group order) + f32 dup rows."""
    from concourse import mybir
    bf16 = mybir.dt.np(mybir.dt.bfloat16)

    pred = np.asarray(pred, dtype=np.float32)
    target = np.asarray(target).astype(np.int64)

    in_maps = []
    rows = ROWS_CORE
    for c in range(N_CORES):
        pc = pred[c * rows:(c + 1) * rows]
        tc_ = target[c * rows:(c + 1) * rows]
        bt = (tc_ >= 2) & (tc_ <= 7)
        dup = np.take_along_axis(pc, tc_[:, None], axis=1)[:, 0] \
            + np.float32(BT_SHIFT) * bt
        lg = pc[:, PERM].astype(bf16)
        if PAD_PER_CORE:
            lg = np.concatenate(
                [lg, np.zeros((PAD_PER_CORE, C), bf16)], axis=0)
            dup = np.concatenate(
                [dup.astype(np.float32),
                 np.zeros(PAD_PER_CORE, np.float32)], axis=0)
        m = {}
        off = 0
        for i, wi in enumerate(TILE_WS):
            n_i = P * wi
            m[f"log{i}"] = np.ascontiguousarray(
                lg[off:off + n_i].reshape(P, wi * C))
            m[f"dup{i}"] = np.ascontiguousarray(
                dup[off:off + n_i].astype(np.float32).reshape(P, wi))
            off += n_i
        in_maps.append(m)
    return in_maps


def kernel(pred, target):
    from concourse.bass_utils import run_bass_kernel_spmd

    nc = _get_nc()
    in_maps = _host_prep(pred, target)
    res = run_bass_kernel_spmd(nc, in_maps, core_ids=list(range(N_CORES)))

    target = np.asarray(target).astype(np.int64)
    nbt = int(((target >= 2) & (target <= 7)).sum())

    sum_lg = 0.0
    sum_g = 0.0
    sum_mm = 0.0
    for c in range(N_CORES):
        o = res.results[c]["out"].astype(np.float64)
        sum_lg += o[:, 0].sum()
        sum_g += o[:, 1].sum()
        sum_mm += o[:, 2].sum()

    # padded rows: logits = 0 -> lse = ln(10); d6 = 0 counts via the bt=0
    # branch; dup = 0 contributes nothing to sum_g.
    sum_lg -= N_CORES * PAD_PER_CORE * np.log(10.0)
    sum_mm -= N_CORES * PAD_PER_CORE
    sum_g -= BT_SHIFT * nbt

    ce = (sum_lg - sum_g) / N
    bce = 100.0 * sum_mm / N
    return np.float32(ce + bce)


# revision 4
# speedup vs baseline: 1.4288x; 1.1233x over previous
"""Trainium2 Bass kernel for nn_CustomLoss (CrossEntropy + binary-remap BCE).

loss = mean_i[ logsumexp(pred_i) - pred_i[t_i] ]
     + 100 * mean_i[ 1{ LUT[argmax(pred_i)] != LUT[t_i] } ]

with LUT = [0,0,1,1,1,1,1,1,0,0]  (LUT[j] = 1 iff 2 <= j <= 7).

Sharding: data-parallel over the batch axis across 8 NeuronCores; each core
returns 3 per-partition partial sums which the host folds into the scalar.

Layout: the host packs logits CLASS-MAJOR per tile — log[i] is [P, 10, w]
bf16 with class c at columns [c*w, (c+1)*w) — in LUT-group order
[2,3,4,5,6,7 | 0,1,8,9].  Every per-row reduction then becomes a tree of
large CONTIGUOUS [P, k*w] tensor_tensor ops, which hit the DVE's 2x bf16
packing (~0.65 ns/col vs ~1.06 for tensor_reduce, which never packs) and
split cleanly across DVE and GPSIMD (Pool does add/mult only).

  DMA   : log bf16 [P, 10, w] (20 B/row) + dup f32 [P, w] (4 B/row) where
          dup = pred[t] + 64*bt.  24 B/row total.
  ACT   : E = exp(log) bf16 contiguous; Ln of the bf16 row-sums with
          per-partition accumulate -> sum of logsumexp.
  DVE   : h5L = E[0:3w]+E[5w:8w]; max tree for m6 (classes 0..5) and m4
          (classes 6..9); d6 = m6 - m4 (f32); dup accumulate
          (tensor_scalar); COUNT_MM_ANT 2-port custom op:
          mismatch = bt ? (d6 < 0) : (d6 >= 0), accum -> count.
          argmax in exp space is exact (exp monotone); bf16 ties resolve
          like argmax-first-index because the g6 group compares first.
  GPSIMD: h5R = E[3w:5w]+E[8w:10w]; t2 = h5[0:2w]+h5[2w:4w];
          t1 = t2[0:w]+t2[w:2w]; s = t1 + h5[4w:5w]  (the add tree tail).

The bt flag rides in the dup stream (+64 when LUT[t] = 1, threshold 32);
the 64*sum(bt) is target-derived and subtracted on the host.  Padded rows
have all-zero logits (lse = ln 10, d6 = 0 -> counts once via the bt=0
branch) and dup = 0; both pad contributions are exact host-side constants.
Tile widths are EVEN so every [c*w] slice is 4-byte aligned (bf16 2x mode
requires it).
"""

import numpy as np

# ---------------------------------------------------------------- constants
N = 2_000_000
C = 10
N_CORES = 8
P = 128
# variable tile widths: small first tile to start compute early, small last
# tile to shorten the drain; sum * P = padded rows per core.  All EVEN.
TILE_WS = [122, 490, 490, 490, 364]
W_SUM = sum(TILE_WS)          # 1,956
ROWS_CORE_PAD = P * W_SUM     # 250,368
ROWS_CORE = N // N_CORES      # 250,000
PAD_PER_CORE = ROWS_CORE_PAD - ROWS_CORE  # 368
# class order: LUT=1 group (2..7) first, LUT=0 group (0,1,8,9) second
PERM = [2, 3, 4, 5, 6, 7, 0, 1, 8, 9]
BT_SHIFT = 64.0
BT_THRESH = 32.0

_CACHE = {}


# ------------------------------------------------------- custom DVE op
def _register_custom_ops():
    """Register the 2-port mismatch-count op (idempotent)."""
    import concourse.dve_ops as dve_ops
    from concourse.dve_spec import (
        Spec, Src0, Src1, Zero, select, lower, AluOp, Bin, C0,
    )
    from concourse.dve_uop import DveOpSpec

    def _get(name):
        for op in dve_ops.OPS:
            if op.name == name:
                return op
        return None

    def _register(name, spec):
        existing = _get(name)
        if existing is not None:
            return existing
        opcode = dve_ops._CUSTOM_DVE_ROW_BASE + len(dve_ops.OPS)
        assert opcode < 0x20, "custom DVE opcode rows exhausted"
        from concourse.dve_ops import has_src1
        shas = {}
        for ver in ("v3", "v4"):
            uops = lower(spec, ver=ver)
            tmp = DveOpSpec(name=name, opcode=opcode, uops=uops,
                            rd1_en=has_src1(spec))
            shas[ver] = tmp.sha(ver)
        op = dve_ops.DveOp(name, spec, subdim=False, uops_sha=shas)
        dve_ops.OPS.append(op)
        dve_ops._SUB_OPCODE_FOR_NAME[name] = opcode
        dve_ops.CUSTOM_DVE_SPECS[name] = spec
        return op

    # COUNT_MM: in0 = d6 = m6 - m4, in1 = dup (>= s0 encodes bt = 1).
    # mismatch = bt ? (d6 < 0) : (d6 >= 0); accum counts mismatches.
    def _count_ref(in0, in1, s0, s1, imm2):
        p = in0.shape[0]
        d6 = np.asarray(in0, np.float32).reshape(p, -1)
        dup = np.asarray(in1, np.float32).reshape(p, -1)
        s0v = np.asarray(s0, np.float32).reshape(p, 1) \
            if isinstance(s0, np.ndarray) else np.float32(s0)
        bt = dup >= s0v
        mm = np.where(bt, d6 < 0, d6 >= 0).astype(np.float32)
        acc = mm.sum(axis=1, dtype=np.float64).astype(np.float32)[:, None]
        return mm.reshape(in0.shape), acc

    body = select(Src1 >= C0,
                  Bin(AluOp.IS_LT, Src0, Zero),
                  Bin(AluOp.IS_GE, Src0, Zero))
    count_spec = Spec(
        body=body,
        accum=AluOp.ADD,
        accum_init=Zero,
        reference=_count_ref,
    )
    cop = _register("COUNT_MM_ANT", count_spec)
    return cop


# ------------------------------------------------------------- device build
def _build_nc(tile_ws=None):
    import concourse.bass as bass
    import concourse.tile as tile
    from concourse import bacc, mybir

    cop = _register_custom_ops()
    f32 = mybir.dt.float32
    bf16 = mybir.dt.bfloat16
    A = mybir.ActivationFunctionType
    X = mybir.AxisListType.X
    alu = mybir.AluOpType

    if tile_ws is None:
        tile_ws = TILE_WS
    tiles = len(tile_ws)
    nc = bacc.Bacc("TRN2", target_bir_lowering=False, debug=False,
                   num_devices=N_CORES)
    log_ds = [
        nc.dram_tensor(f"log{i}", [P, wi * C], bf16,
                       kind="ExternalInput").ap()
        for i, wi in enumerate(tile_ws)
    ]
    dup_ds = [
        nc.dram_tensor(f"dup{i}", [P, wi], f32,
                       kind="ExternalInput").ap()
        for i, wi in enumerate(tile_ws)
    ]
    out_d = nc.dram_tensor("out", [P, 3], f32, kind="ExternalOutput").ap()

    with tile.TileContext(nc) as tc:
        with (
            tc.tile_pool(name="io", bufs=3) as io,
            tc.tile_pool(name="ep", bufs=3) as ep,
            tc.tile_pool(name="mp", bufs=2) as mp,
            tc.tile_pool(name="cp", bufs=1) as cp,
        ):
            acc_all = cp.tile([P, 3, tiles], f32)
            acc_lg = acc_all[:, 0, :]
            acc_g = acc_all[:, 1, :]
            acc_mm = acc_all[:, 2, :]

            for i in range(tiles):
                w = tile_ws[i]
                lt = io.tile([P, w * C], bf16, tag="log")
                nc.sync.dma_start(lt[:], log_ds[i])
                dt = io.tile([P, w], f32, tag="dup")
                nc.sync.dma_start(dt[:], dup_ds[i])

                # E = exp(logits), class-major [P, 10, w] flattened
                et = ep.tile([P, w * C], bf16, tag="E")
                nc.scalar.activation(et[:], lt[:], A.Exp)
                ev = et[:]

                # ---- row-sum tree (adds: DVE takes the big L1 half,
                #      GPSIMD the rest)
                h5 = mp.tile([P, 5 * w], bf16, tag="h5")
                nc.vector.tensor_tensor(
                    h5[:, 0:3 * w], ev[:, 0:3 * w], ev[:, 5 * w:8 * w],
                    op=alu.add)
                nc.gpsimd.tensor_tensor(
                    h5[:, 3 * w:5 * w], ev[:, 3 * w:5 * w],
                    ev[:, 8 * w:10 * w], op=alu.add)
                t2 = mp.tile([P, 2 * w], bf16, tag="t2")
                nc.gpsimd.tensor_tensor(
                    t2[:], h5[:, 0:2 * w], h5[:, 2 * w:4 * w], op=alu.add)
                s = mp.tile([P, w], bf16, tag="s")
                nc.gpsimd.tensor_tensor(
                    s[:, 0:w], t2[:, 0:w], t2[:, w:2 * w], op=alu.add)
                nc.gpsimd.tensor_tensor(
                    s[:, 0:w], s[:, 0:w], h5[:, 4 * w:5 * w], op=alu.add)

                lnj = mp.tile([P, w], bf16, tag="lnj")
                nc.scalar.activation(lnj[:], s[:], A.Ln,
                                     accum_out=acc_lg[:, i:i + 1])

                # ---- gathered-logit accumulate (DVE tensor_scalar)
                gj = mp.tile([P, w], f32, tag="gj")
                nc.vector.tensor_scalar(gj[:], dt[:], 1.0, 0.0,
                                        op0=alu.mult, op1=alu.add,
                                        accum_out=acc_g[:, i:i + 1])

                # ---- BCE: group max trees in exp space (DVE)
                a3 = mp.tile([P, 3 * w], bf16, tag="a3")
                nc.vector.tensor_tensor(
                    a3[:], ev[:, 0:3 * w], ev[:, 3 * w:6 * w], op=alu.max)
                b1 = mp.tile([P, w], bf16, tag="b1")
                nc.vector.tensor_tensor(
                    b1[:], a3[:, 0:w], a3[:, w:2 * w], op=alu.max)
                m6 = mp.tile([P, w], bf16, tag="m6")
                nc.vector.tensor_tensor(
                    m6[:], b1[:], a3[:, 2 * w:3 * w], op=alu.max)
                c2 = mp.tile([P, 2 * w], bf16, tag="c2")
                nc.vector.tensor_tensor(
                    c2[:], ev[:, 6 * w:8 * w], ev[:, 8 * w:10 * w],
                    op=alu.max)
                m4 = mp.tile([P, w], bf16, tag="m4")
                nc.vector.tensor_tensor(
                    m4[:], c2[:, 0:w], c2[:, w:2 * w], op=alu.max)
                d6 = mp.tile([P, w], f32, tag="d6")
                nc.vector.tensor_tensor(
                    d6[:], m6[:], m4[:], op=alu.subtract)

                cj = mp.tile([P, w], f32, tag="cj")
                nc.vector._custom_dve(
                    cop, out=cj[:], in0=d6[:], in1=dt[:], s0=BT_THRESH,
                    accum_out=acc_mm[:, i:i + 1])

            # ---- final per-partition reduction + store (one fused reduce)
            out_t = cp.tile([P, 3], f32)
            nc.vector.reduce_sum(out_t[:], acc_all[:], axis=X)
            nc.sync.dma_start(out_d[:], out_t[:])

    # Force a single activation table containing both Exp and Ln so the
    # compiler does not ping-pong ACT_TABLE_LOADs.  Table ids are positional,
    # so keep the dict shape and empty the other sets.
    import concourse.bacc as bacc_mod
    from concourse.hw_specs import get_activation_tables
    orig = get_activation_tables(nc.m.arch)
    combined = None
    for k, v in orig.items():
        if (mybir.ActivationFunctionType.Exp in v
                and mybir.ActivationFunctionType.Ln in v):
            combined = k
            break
    if combined is not None:
        patched = {k: (v if k == combined else set()) for k, v in orig.items()}
        saved = bacc_mod.get_activation_tables
        bacc_mod.get_activation_tables = lambda arch: patched
        try:
            nc.compile()
        finally:
            bacc_mod.get_activation_tables = saved
    else:
        nc.compile()
    return nc


def _get_nc():
    if "nc" not in _CACHE:
        _CACHE["nc"] = _build_nc()
    return _CACHE["nc"]


# ------------------------------------------------------------------- host
def _host_prep(pred, target):
    """Shard + pad inputs; pack bf16 class-major logits + f32 dup rows."""
    from concourse import mybir
    bf16 = mybir.dt.np(mybir.dt.bfloat16)

    pred = np.asarray(pred, dtype=np.float32)
    target = np.asarray(target).astype(np.int64)

    in_maps = []
    rows = ROWS_CORE
    for c in range(N_CORES):
        pc = pred[c * rows:(c + 1) * rows]
        tc_ = target[c * rows:(c + 1) * rows]
        bt = (tc_ >= 2) & (tc_ <= 7)
        dup = np.take_along_axis(pc, tc_[:, None], axis=1)[:, 0] \
            + np.float32(BT_SHIFT) * bt
        lg = pc[:, PERM].astype(bf16)
        if PAD_PER_CORE:
            lg = np.concatenate(
                [lg, np.zeros((PAD_PER_CORE, C), bf16)], axis=0)
            dup = np.concatenate(
                [dup.astype(np.float32),
                 np.zeros(PAD_PER_CORE, np.float32)], axis=0)
        m = {}
        off = 0
        for i, wi in enumerate(TILE_WS):
            n_i = P * wi
            # class-major: [P, wi, 10] -> [P, 10, wi]
            m[f"log{i}"] = np.ascontiguousarray(
                lg[off:off + n_i].reshape(P, wi, C).transpose(0, 2, 1)
            ).reshape(P, wi * C)
            m[f"dup{i}"] = np.ascontiguousarray(
                dup[off:off + n_i].astype(np.float32).reshape(P, wi))
            off += n_i
        in_maps.append(m)
    return in_maps


def kernel(pred, target):
    from concourse.bass_utils import run_bass_kernel_spmd

    nc = _get_nc()
    in_maps = _host_prep(pred, target)
    res = run_bass_kernel_spmd(nc, in_maps, core_ids=list(range(N_CORES)))

    target = np.asarray(target).astype(np.int64)
    nbt = int(((target >= 2) & (target <= 7)).sum())

    sum_lg = 0.0
    sum_g = 0.0
    sum_mm = 0.0
    for c in range(N_CORES):
        o = res.results[c]["out"].astype(np.float64)
        sum_lg += o[:, 0].sum()
        sum_g += o[:, 1].sum()
        sum_mm += o[:, 2].sum()

    # padded rows: logits = 0 -> lse = ln(10); d6 = 0 counts via the bt=0
    # branch; dup = 0 contributes nothing to sum_g.
    sum_lg -= N_CORES * PAD_PER_CORE * np.log(10.0)
    sum_mm -= N_CORES * PAD_PER_CORE
    sum_g -= BT_SHIFT * nbt

    ce = (sum_lg - sum_g) / N
    bce = 100.0 * sum_mm / N
    return np.float32(ce + bce)


# revision 5
# speedup vs baseline: 1.8444x; 1.2909x over previous
"""Trainium2 Bass kernel for nn_CustomLoss (CrossEntropy + binary-remap BCE).

loss = mean_i[ logsumexp(pred_i) - pred_i[t_i] ]
     + 100 * mean_i[ 1{ LUT[argmax(pred_i)] != LUT[t_i] } ]

with LUT = [0,0,1,1,1,1,1,1,0,0]  (LUT[j] = 1 iff 2 <= j <= 7).

Sharding: data-parallel over the batch axis across 8 NeuronCores; each core
returns 3 per-partition partial sums which the host folds into the scalar.

Layout: the host packs logits CLASS-MAJOR per tile — log[i] is [P, 10, w]
bf16 with class c at columns [c*w, (c+1)*w) — in LUT-group order
[2,3,4,5,6,7 | 0,1,8,9].  Every per-row reduction then becomes a tree of
large CONTIGUOUS [P, k*w] tensor_tensor ops, which hit the DVE's 2x bf16
packing (~0.65 ns/col vs ~1.06 for tensor_reduce, which never packs) and
split cleanly across DVE and GPSIMD (Pool does add/mult only).

  DMA   : log bf16 [P, 10, w] (20 B/row) + dup f32 [P, w] (4 B/row) where
          dup = pred[t] + 64*bt.  24 B/row total.
  ACT   : E = exp(log) bf16 contiguous; Ln of the bf16 row-sums with
          per-partition accumulate -> sum of logsumexp.
  DVE   : h5L = E[0:3w]+E[5w:8w]; max tree for m6 (classes 0..5) and m4
          (classes 6..9); d6 = m6 - m4 (f32); dup accumulate
          (tensor_scalar); COUNT_MM_ANT 2-port custom op:
          mismatch = bt ? (d6 < 0) : (d6 >= 0), accum -> count.
          argmax in exp space is exact (exp monotone); bf16 ties resolve
          like argmax-first-index because the g6 group compares first.
  GPSIMD: h5R = E[3w:5w]+E[8w:10w]; t2 = h5[0:2w]+h5[2w:4w];
          t1 = t2[0:w]+t2[w:2w]; s = t1 + h5[4w:5w]  (the add tree tail).

The bt flag rides in the dup stream (+64 when LUT[t] = 1, threshold 32);
the 64*sum(bt) is target-derived and subtracted on the host.  Padded rows
have all-zero logits (lse = ln 10, d6 = 0 -> counts once via the bt=0
branch) and dup = 0; both pad contributions are exact host-side constants.
Tile widths are EVEN so every [c*w] slice is 4-byte aligned (bf16 2x mode
requires it).
"""

import numpy as np

# ---------------------------------------------------------------- constants
N = 2_000_000
C = 10
N_CORES = 8
P = 128
# variable tile widths: small first tile to start compute early, small last
# tile to shorten the drain; sum * P = padded rows per core.  All EVEN.
TILE_WS = [122, 490, 490, 490, 364]
W_SUM = sum(TILE_WS)          # 1,956
ROWS_CORE_PAD = P * W_SUM     # 250,368
ROWS_CORE = N // N_CORES      # 250,000
PAD_PER_CORE = ROWS_CORE_PAD - ROWS_CORE  # 368
# class-slot order: [g6a | g4a | g6b | g4b] so that slot k pairs with
# slot k+5 within the same LUT group: (2,5),(3,6),(4,7) in g6, (0,8),(1,9)
# in g4.  One [P,5w] pairwise max then feeds both group max trees.
PERM = [2, 3, 4, 0, 1, 5, 6, 7, 8, 9]
BT_SHIFT = 64.0
BT_THRESH = 32.0

_CACHE = {}


# ------------------------------------------------------- custom DVE op
def _register_custom_ops():
    """Register the 2-port mismatch-count op (idempotent)."""
    import concourse.dve_ops as dve_ops
    from concourse.dve_spec import (
        Spec, Src0, Src1, Zero, select, lower, AluOp, Bin, C0,
    )
    from concourse.dve_uop import DveOpSpec

    def _get(name):
        for op in dve_ops.OPS:
            if op.name == name:
                return op
        return None

    def _register(name, spec):
        existing = _get(name)
        if existing is not None:
            return existing
        opcode = dve_ops._CUSTOM_DVE_ROW_BASE + len(dve_ops.OPS)
        assert opcode < 0x20, "custom DVE opcode rows exhausted"
        from concourse.dve_ops import has_src1
        shas = {}
        for ver in ("v3", "v4"):
            uops = lower(spec, ver=ver)
            tmp = DveOpSpec(name=name, opcode=opcode, uops=uops,
                            rd1_en=has_src1(spec))
            shas[ver] = tmp.sha(ver)
        op = dve_ops.DveOp(name, spec, subdim=False, uops_sha=shas)
        dve_ops.OPS.append(op)
        dve_ops._SUB_OPCODE_FOR_NAME[name] = opcode
        dve_ops.CUSTOM_DVE_SPECS[name] = spec
        return op

    # COUNT_MM: in0 = d6 = m6 - m4, in1 = dup (>= s0 encodes bt = 1).
    # mismatch = bt ? (d6 < 0) : (d6 >= 0); accum counts mismatches.
    def _count_ref(in0, in1, s0, s1, imm2):
        p = in0.shape[0]
        d6 = np.asarray(in0, np.float32).reshape(p, -1)
        dup = np.asarray(in1, np.float32).reshape(p, -1)
        s0v = np.asarray(s0, np.float32).reshape(p, 1) \
            if isinstance(s0, np.ndarray) else np.float32(s0)
        bt = dup >= s0v
        mm = np.where(bt, d6 < 0, d6 >= 0).astype(np.float32)
        acc = mm.sum(axis=1, dtype=np.float64).astype(np.float32)[:, None]
        return mm.reshape(in0.shape), acc

    body = select(Src1 >= C0,
                  Bin(AluOp.IS_LT, Src0, Zero),
                  Bin(AluOp.IS_GE, Src0, Zero))
    count_spec = Spec(
        body=body,
        accum=AluOp.ADD,
        accum_init=Zero,
        reference=_count_ref,
    )
    cop = _register("COUNT_MM_ANT", count_spec)
    return cop


# ------------------------------------------------------------- device build
def _build_nc(tile_ws=None):
    import concourse.bass as bass
    import concourse.tile as tile
    from concourse import bacc, mybir

    cop = _register_custom_ops()
    f32 = mybir.dt.float32
    bf16 = mybir.dt.bfloat16
    A = mybir.ActivationFunctionType
    X = mybir.AxisListType.X
    alu = mybir.AluOpType

    if tile_ws is None:
        tile_ws = TILE_WS
    tiles = len(tile_ws)
    nc = bacc.Bacc("TRN2", target_bir_lowering=False, debug=False,
                   num_devices=N_CORES)
    log_ds = [
        nc.dram_tensor(f"log{i}", [P, wi * C], bf16,
                       kind="ExternalInput").ap()
        for i, wi in enumerate(tile_ws)
    ]
    dup_ds = [
        nc.dram_tensor(f"dup{i}", [P, wi], f32,
                       kind="ExternalInput").ap()
        for i, wi in enumerate(tile_ws)
    ]
    out_d = nc.dram_tensor("out", [P, 3], f32, kind="ExternalOutput").ap()

    with tile.TileContext(nc) as tc:
        with (
            tc.tile_pool(name="io", bufs=3) as io,
            tc.tile_pool(name="ep", bufs=3) as ep,
            tc.tile_pool(name="mp", bufs=2) as mp,
            tc.tile_pool(name="cp", bufs=1) as cp,
        ):
            acc_all = cp.tile([P, 3, tiles], f32)
            acc_lg = acc_all[:, 0, :]
            acc_g = acc_all[:, 1, :]
            acc_mm = acc_all[:, 2, :]

            for i in range(tiles):
                w = tile_ws[i]
                lt = io.tile([P, w * C], bf16, tag="log")
                nc.sync.dma_start(lt[:], log_ds[i])
                dt = io.tile([P, w], f32, tag="dup")
                nc.sync.dma_start(dt[:], dup_ds[i])

                # E = exp(logits), class-major [P, 10, w] flattened
                et = ep.tile([P, w * C], bf16, tag="E")
                nc.scalar.activation(et[:], lt[:], A.Exp)
                ev = et[:]

                # ---- row-sum tree (all DVE; GPSIMD stays idle because any
                #      GPSIMD op halves 2-port DVE throughput: shared port)
                h5 = mp.tile([P, 5 * w], bf16, tag="h5")
                nc.vector.tensor_tensor(
                    h5[:], ev[:, 0:5 * w], ev[:, 5 * w:10 * w], op=alu.add)
                t2 = mp.tile([P, 2 * w], bf16, tag="t2")
                nc.vector.tensor_tensor(
                    t2[:], h5[:, 0:2 * w], h5[:, 2 * w:4 * w], op=alu.add)
                s = mp.tile([P, w], bf16, tag="s")
                nc.vector.tensor_tensor(
                    s[:], t2[:, 0:w], t2[:, w:2 * w], op=alu.add)
                nc.vector.tensor_tensor(
                    s[:], s[:], h5[:, 4 * w:5 * w], op=alu.add)

                lnj = mp.tile([P, w], bf16, tag="lnj")
                nc.scalar.activation(lnj[:], s[:], A.Ln,
                                     accum_out=acc_lg[:, i:i + 1])

                # ---- gathered-logit accumulate (ACT Copy with accum)
                gj = mp.tile([P, w], bf16, tag="gj")
                nc.scalar.activation(gj[:], dt[:], A.Copy,
                                     accum_out=acc_g[:, i:i + 1])

                # ---- BCE: group max trees in exp space (DVE)
                mx = mp.tile([P, 5 * w], bf16, tag="mx")
                nc.vector.tensor_tensor(
                    mx[:], ev[:, 0:5 * w], ev[:, 5 * w:10 * w], op=alu.max)
                n1 = mp.tile([P, w], bf16, tag="n1")
                nc.vector.tensor_tensor(
                    n1[:], mx[:, 0:w], mx[:, w:2 * w], op=alu.max)
                m6 = mp.tile([P, w], bf16, tag="m6")
                nc.vector.tensor_tensor(
                    m6[:], n1[:], mx[:, 2 * w:3 * w], op=alu.max)
                m4 = mp.tile([P, w], bf16, tag="m4")
                nc.vector.tensor_tensor(
                    m4[:], mx[:, 3 * w:4 * w], mx[:, 4 * w:5 * w],
                    op=alu.max)
                d6 = mp.tile([P, w], f32, tag="d6")
                nc.vector.tensor_tensor(
                    d6[:], m6[:], m4[:], op=alu.subtract)

                cj = mp.tile([P, w], f32, tag="cj")
                nc.vector._custom_dve(
                    cop, out=cj[:], in0=d6[:], in1=dt[:], s0=BT_THRESH,
                    accum_out=acc_mm[:, i:i + 1])

            # ---- final per-partition reduction + store (one fused reduce)
            out_t = cp.tile([P, 3], f32)
            nc.vector.reduce_sum(out_t[:], acc_all[:], axis=X)
            nc.sync.dma_start(out_d[:], out_t[:])

    # Force a single activation table containing both Exp and Ln so the
    # compiler does not ping-pong ACT_TABLE_LOADs.  Table ids are positional,
    # so keep the dict shape and empty the other sets.
    import concourse.bacc as bacc_mod
    from concourse.hw_specs import get_activation_tables
    orig = get_activation_tables(nc.m.arch)
    combined = None
    for k, v in orig.items():
        if (mybir.ActivationFunctionType.Exp in v
                and mybir.ActivationFunctionType.Ln in v):
            combined = k
            break
    if combined is not None:
        patched = {k: (v if k == combined else set()) for k, v in orig.items()}
        saved = bacc_mod.get_activation_tables
        bacc_mod.get_activation_tables = lambda arch: patched
        try:
            nc.compile()
        finally:
            bacc_mod.get_activation_tables = saved
    else:
        nc.compile()
    return nc


def _get_nc():
    if "nc" not in _CACHE:
        _CACHE["nc"] = _build_nc()
    return _CACHE["nc"]


# ------------------------------------------------------------------- host
def _host_prep(pred, target):
    """Shard + pad inputs; pack bf16 class-major logits + f32 dup rows."""
    from concourse import mybir
    bf16 = mybir.dt.np(mybir.dt.bfloat16)

    pred = np.asarray(pred, dtype=np.float32)
    target = np.asarray(target).astype(np.int64)

    in_maps = []
    rows = ROWS_CORE
    for c in range(N_CORES):
        pc = pred[c * rows:(c + 1) * rows]
        tc_ = target[c * rows:(c + 1) * rows]
        bt = (tc_ >= 2) & (tc_ <= 7)
        dup = np.take_along_axis(pc, tc_[:, None], axis=1)[:, 0] \
            + np.float32(BT_SHIFT) * bt
        lg = pc[:, PERM].astype(bf16)
        if PAD_PER_CORE:
            lg = np.concatenate(
                [lg, np.zeros((PAD_PER_CORE, C), bf16)], axis=0)
            dup = np.concatenate(
                [dup.astype(np.float32),
                 np.zeros(PAD_PER_CORE, np.float32)], axis=0)
        m = {}
        off = 0
        for i, wi in enumerate(TILE_WS):
            n_i = P * wi
            # class-major: [P, wi, 10] -> [P, 10, wi]
            m[f"log{i}"] = np.ascontiguousarray(
                lg[off:off + n_i].reshape(P, wi, C).transpose(0, 2, 1)
            ).reshape(P, wi * C)
            m[f"dup{i}"] = np.ascontiguousarray(
                dup[off:off + n_i].astype(np.float32).reshape(P, wi))
            off += n_i
        in_maps.append(m)
    return in_maps


def kernel(pred, target):
    from concourse.bass_utils import run_bass_kernel_spmd

    nc = _get_nc()
    in_maps = _host_prep(pred, target)
    res = run_bass_kernel_spmd(nc, in_maps, core_ids=list(range(N_CORES)))

    target = np.asarray(target).astype(np.int64)
    nbt = int(((target >= 2) & (target <= 7)).sum())

    sum_lg = 0.0
    sum_g = 0.0
    sum_mm = 0.0
    for c in range(N_CORES):
        o = res.results[c]["out"].astype(np.float64)
        sum_lg += o[:, 0].sum()
        sum_g += o[:, 1].sum()
        sum_mm += o[:, 2].sum()

    # padded rows: logits = 0 -> lse = ln(10); d6 = 0 counts via the bt=0
    # branch; dup = 0 contributes nothing to sum_g.
    sum_lg -= N_CORES * PAD_PER_CORE * np.log(10.0)
    sum_mm -= N_CORES * PAD_PER_CORE
    sum_g -= BT_SHIFT * nbt

    ce = (sum_lg - sum_g) / N
    bce = 100.0 * sum_mm / N
    return np.float32(ce + bce)
